# revision 5
# baseline (speedup 1.0000x reference)
"""Trainium2 Bass kernel for nn_BinaryTokenClassificationModel (segment_reduce).

Math: the reference pools token embeddings into word embeddings (mean over
contiguous runs of equal word ids), then computes
    logits[b,s,t] = src_pooled[b,s] @ w_src + tgt_pooled[b,t] @ w_tgt + b.
Because the classifier is linear, pooling and projection commute:
    u[t]    = tok_h[t] @ w_blk(t)              (per-token scalar projection)
    logits[s, t] = sum_t atw_src[t, s] u[t] + sum_t atw_tgt[t, t'] u[t] + b
where atw is the 1/count-weighted segment membership matrix; the [S, T]
PSUM tile accumulates segment-reduce + outer-sum in one matmul per chunk
(the scalar u rides a stride-0 broadcast operand).
Data-parallel over batch: core i handles batch row i. No collectives.

Engine mapping (GpSimd and ACT-compute deliberately unused -- the GpSimd
pool library costs ~12us of load/drain on HW, and any ACT op inserts a
1.3us ACT_TABLE_LOAD in front of the scalar engine's DMA issues):
  host   : tok_h cast to bf16 (halves DMA bytes; tolerance is 2e-2),
           membership matrices built in numpy and packed next to the token
           columns so each chunk is ONE contiguous DMA, W pre-concatenated
  DMA    : W row broadcast straight from DRAM to all 128 partitions
           (stride-0 source), bias likewise; chunks split across the sync
           and scalar queues to halve per-engine issue serialization
  DVE    : u = tok . w fused multiply-reduce per chunk, fp32->bf16 cast of
           u, final bias-add+copy of the PSUM tile
  PE     : one [128,*]x[128,128] matmul per chunk into the [S,T] PSUM tile
"""

import functools

import numpy as np
import ml_dtypes

import concourse.bacc as bacc
import concourse.mybir as mybir
from concourse.bass_utils import run_bass_kernel_spmd
from concourse.tile import TileContext

# Problem geometry (hardcoded per spec)
B = 8
L_SRC = 256
L_TGT = 256
L = L_SRC + L_TGT  # 512
H = 768
P = 128            # SBUF partitions / tokens per chunk
NCHUNK = L // P    # 4
N_SRC_CHUNKS = L_SRC // P  # 2
N_CORES = 8
F32 = mybir.dt.float32
BF16 = mybir.dt.bfloat16
NPBF16 = ml_dtypes.bfloat16


# ---------------------------------------------------------------------------
# Host-side segment bookkeeping (exact mirror of reference._pool_words)
# ---------------------------------------------------------------------------

def _segments(combined_wid, attention_mask, n_words):
    """Per-token dense run ids exactly as the reference computes them."""
    valid = (attention_mask > 0) & (combined_wid >= 0)  # [B, L]
    prev_wid = np.concatenate(
        [np.full((combined_wid.shape[0], 1), -2, dtype=combined_wid.dtype),
         combined_wid[:, :-1]], axis=1)
    prev_valid = np.concatenate(
        [np.zeros((valid.shape[0], 1), dtype=bool), valid[:, :-1]], axis=1)
    new_run = valid & ((combined_wid != prev_wid) | (~prev_valid))
    run_id = np.cumsum(new_run.astype(np.int64), axis=1) - 1  # [B, L]
    seg = np.where(valid, run_id, n_words)  # n_words = dummy slot
    return seg, valid


def _seg_weights(seg, valid, n_words):
    """1/max(count,1) weight for each token's segment (0 for invalid)."""
    Bv, Lv = seg.shape
    wgt = np.zeros((Bv, Lv), dtype=np.float32)
    for b in range(Bv):
        counts = np.bincount(seg[b][valid[b]], minlength=Lv + 1).astype(np.float32)
        inv = 1.0 / np.maximum(counts, 1.0)
        wgt[b] = np.where(valid[b] & (seg[b] < n_words), inv[np.minimum(seg[b], Lv)], 0.0)
    return wgt


# ---------------------------------------------------------------------------
# Device kernel
# ---------------------------------------------------------------------------

def _emit(nc, tc, S, T, block_ok, wb_np, b_val):
    """block_ok fast path: src tokens only map to word rows [0,S), tgt
    tokens only to [S,S+T) -> each chunk's membership is [128, P] and each
    chunk does ONE reduce.  General path: membership is [128, S+T] and each
    chunk reduces against both weight halves.

    wb_np ([128, 2H] bf16, W pre-broadcast) and b_val (python float bias)
    are baked into the NEFF: wb as an inline DRAM constant (one fast
    striped DMA, no broadcast machinery on device), the bias as an
    immediate on the final DVE add."""
    NW = S + T
    AW = P if block_ok else NW
    CW = H + AW  # packed chunk width: tok columns then membership columns
    chunks = nc.declare_dram_parameter("chunks", [NCHUNK, P, CW], BF16,
                                       isOutput=False)
    wsrc_c = nc.inline_tensor(wb_np[:, 0:H], name="wsrc_c")
    wtgt_c = nc.inline_tensor(wb_np[:, H:2 * H], name="wtgt_c")
    out = nc.declare_dram_parameter("out", [S, T], F32, isOutput=True)

    with (
        tc.tile_pool(name="const", bufs=1) as cpool,
        tc.tile_pool(name="toks", bufs=1) as tpool,
        tc.tile_pool(name="prods", bufs=2) as ppool,
        tc.tile_pool(name="psum", bufs=1, space="PSUM") as pspool,
    ):
        # pre-broadcast weights from NEFF-embedded constants (a stride-0
        # broadcast DMA measured ~6us; these are ordinary striped DMAs).
        # wsrc leads the sync queue (gates reduce 0); wtgt is only needed
        # ~2us later so it rides the scalar queue, whose first transfer is
        # delayed behind ACT_TABLE_LOAD.
        wb_sb = cpool.tile([P, 2 * H], BF16)
        nc.sync.dma_start(out=wb_sb[:, 0:H], in_=wsrc_c[:])
        nc.scalar.dma_start(out=wb_sb[:, H:2 * H], in_=wtgt_c[:])

        n_u = NCHUNK if block_ok else 2 * NCHUNK
        u_sb = cpool.tile([P, n_u], F32)
        u_bf = cpool.tile([P, n_u], BF16)
        psum_out = pspool.tile([S, T], F32)

        for c in range(NCHUNK):
            ch = tpool.tile([P, CW], BF16, name=f"chunk{c}")
            eng = nc.sync if c < NCHUNK - 1 else nc.scalar
            eng.dma_start(out=ch[:], in_=chunks[c])
            tok_c = ch[:, 0:H]
            atw_c = ch[:, H:CW]

            if block_ok:
                is_src = c < N_SRC_CHUNKS
                jobs = [(0 if is_src else 1, is_src,
                         atw_c[:, 0:(S if is_src else T)], c)]
            else:
                jobs = [(0, True, atw_c[:, 0:S], 2 * c),
                        (1, False, atw_c[:, S:NW], 2 * c + 1)]

            for half, is_src, atw_ap, ui in jobs:
                prod = ppool.tile([P, H], BF16, name=f"prod{ui % 2}")
                nc.vector.affine_mul_reduce(
                    out=prod[:], accum_out=u_sb[:, ui:ui + 1], in0=tok_c,
                    in1=wb_sb[:, half * H:(half + 1) * H], scale=1.0, bias=0.0)
                # last chunk's cast rides DVE (shorter serial tail than the
                # ~300ns ACT hop); earlier ones go to ACT off the DVE chain
                if ui == n_u - 1:
                    nc.vector.tensor_copy(u_bf[:, ui:ui + 1], u_sb[:, ui:ui + 1])
                else:
                    nc.scalar.copy(out=u_bf[:, ui:ui + 1], in_=u_sb[:, ui:ui + 1])
                ub = u_bf[:, ui:ui + 1]
                first = ui == 0
                last = ui == n_u - 1
                if is_src:
                    nc.tensor.matmul(psum_out[:], atw_ap, ub.broadcast_to([P, T]),
                                     start=first, stop=last, skip_group_check=True)
                else:
                    nc.tensor.matmul(psum_out[:], ub.broadcast_to([P, S]), atw_ap,
                                     start=first, stop=last, skip_group_check=True)

        # bias-add + store in row halves so the two DMA issues (and the
        # two 32KB transfers) run on both queues concurrently
        out_sb = cpool.tile([S, T], F32)
        hS = S // 2
        nc.vector.tensor_scalar_add(out_sb[0:hS, :], psum_out[0:hS, :], float(b_val))
        nc.sync.dma_start(out=out[0:hS, :], in_=out_sb[0:hS, :])
        nc.vector.tensor_scalar_add(out_sb[hS:S, :], psum_out[hS:S, :], float(b_val))
        nc.scalar.dma_start(out=out[hS:S, :], in_=out_sb[hS:S, :])


@functools.lru_cache(maxsize=4)
def _build(S, T, block_ok, wb_bytes, b_val):
    wb_np = np.frombuffer(wb_bytes, dtype=NPBF16).reshape(P, 2 * H)
    nc = bacc.Bacc("TRN2", debug=False, num_devices=N_CORES)
    with TileContext(nc) as tc:
        _emit(nc, tc, S, T, block_ok, wb_np, b_val)
    nc.compile()
    return nc


# ---------------------------------------------------------------------------
# Host wrapper
# ---------------------------------------------------------------------------

def _prep(inputs):
    tok_h = np.ascontiguousarray(np.asarray(inputs["tok_h"], dtype=np.float32))
    mask = np.asarray(inputs["attention_mask"])
    swid = np.asarray(inputs["source_word_ids"])
    twid = np.asarray(inputs["target_word_ids"])
    W = np.asarray(inputs["W"], dtype=np.float32)
    b = np.asarray(inputs["b"], dtype=np.float32)
    S = int(np.asarray(inputs["S"]))
    T = int(np.asarray(inputs["T"]))

    Bv, Lv, Hv = tok_h.shape
    assert (Bv, Lv, Hv) == (B, L, H), f"unexpected tok_h shape {tok_h.shape}"
    assert swid.shape == (B, L_SRC) and twid.shape == (B, L_TGT)
    assert S <= P and T <= P

    NW = S + T
    combined = np.concatenate([swid, twid], axis=1).astype(np.int64)
    seg, valid = _segments(combined, mask, NW)
    wgt = _seg_weights(seg, valid, NW)

    src_tok_seg = seg[:, :L_SRC][valid[:, :L_SRC]]
    tgt_tok_seg = seg[:, L_SRC:][valid[:, L_SRC:]]
    block_ok = bool(
        (src_tok_seg < S).all()
        and (tgt_tok_seg >= S).all() and (tgt_tok_seg < NW).all()
    )

    wrow_bf = np.concatenate([W[:H, 0], W[H:2 * H, 0]]).reshape(1, 2 * H).astype(NPBF16)
    wb_np = np.ascontiguousarray(np.broadcast_to(wrow_bf, (P, 2 * H)))
    b_val = float(b.reshape(-1)[0])

    AW = P if block_ok else NW
    CW = H + AW
    tidx = np.arange(L)
    tok_bf = tok_h.astype(NPBF16)
    in_maps = []
    for bi in range(B):
        atw_f = np.zeros((L, AW), dtype=np.float32)
        segb = seg[bi]
        ok = valid[bi] & (segb < NW)
        if block_ok:
            col = np.where(tidx < L_SRC, segb, segb - S)
        else:
            col = segb
        atw_f[tidx[ok], col[ok]] = wgt[bi][ok]
        packed = np.empty((NCHUNK, P, CW), dtype=NPBF16)
        packed[:, :, 0:H] = tok_bf[bi].reshape(NCHUNK, P, H)
        packed[:, :, H:CW] = atw_f.astype(NPBF16).reshape(NCHUNK, P, AW)
        in_maps.append({"chunks": packed})
    return S, T, block_ok, wb_np, b_val, in_maps


def kernel(**inputs):
    S, T, block_ok, wb_np, b_val, in_maps = _prep(inputs)
    nc = _build(S, T, block_ok, wb_np.tobytes(), b_val)
    res = run_bass_kernel_spmd(nc, in_maps, core_ids=list(range(N_CORES)))
    return np.stack([res.results[i]["out"] for i in range(B)], axis=0)
